# revision 27
# baseline (speedup 1.0000x reference)
"""CannyNet Trainium2 Bass kernel.

kernel(**inputs) takes the complete unsharded inputs (img [1,3,2048,2048],
threshold [1], gauss/sobel/dir filter tensors) and returns the reference
tuple (blurred_img, grad_mag, orient, thin_edges, thresholded,
early_threshold) computed on 8 NeuronCores.

Sharding: H is split spatially across the 8 cores (256 rows each).  Each
core receives a zero-padded slab of its rows (+4 halo rows, +4 halo cols)
and processes three 86-row "frames" that are fully self-contained: the
vertical convolutions are band matmuls on the tensor engine (the band
matrices encode the frame offset, the gaussian/sobel taps, and the exact
zero-padding behaviour of the reference at the global image edges), while
horizontal convolutions and pointwise math run on the vector, scalar and
gpsimd engines.  The per-pixel orientation sector (for NMS and the orient
output) is computed with sign tests against the 8 reference sector
boundaries instead of atan2.
"""

import numpy as np

try:  # the grading env has concourse importable; fall back to container path
    import concourse.bass as _b  # noqa: F401
except ImportError:  # pragma: no cover
    import sys
    for _p in ("/opt/trn_rl_repo",):
        if _p not in sys.path:
            sys.path.insert(0, _p)

import concourse.bass as bass
import concourse.bacc as bacc
import concourse.mybir as mybir
import concourse.tile as tile
from concourse.bass_utils import run_bass_kernel_spmd


# ----------------------------------------------------------------------------
# custom fused DVE ops (registered into the per-NEFF DVE table)
# ----------------------------------------------------------------------------
from concourse import dve_ops as _dve_ops
from concourse.dve_spec import (Spec as _Spec, Src0 as _S0, Src1 as _S1,
                                C0 as _C0, C1 as _C1, Zero as _Z,
                                select as _sel, sq as _sq, maxx as _maxx,
                                lower as _dve_lower, _has_src1 as _has_src1)
from concourse.dve_uop import DveOpSpec as _DveOpSpec
import numpy as _np


def _register_op(name, spec, subdim=False):
    for op in _dve_ops.OPS:
        if op.name == name:
            return op
    opcode = _dve_ops._CUSTOM_DVE_ROW_BASE + len(_dve_ops.OPS)
    assert opcode < 0x20, "custom DVE op row overflow"
    shas = {}
    for ver in ("v3", "v4"):
        s = _DveOpSpec(name=name, opcode=opcode, uops=_dve_lower(spec, ver=ver),
                       rd1_en=_has_src1(spec))
        shas[ver] = s.sha(ver)
    op = _dve_ops.DveOp(name, spec, subdim, shas)
    _dve_ops.OPS.append(op)
    _dve_ops._SUB_OPCODE_FOR_NAME[name] = opcode
    _dve_ops.CUSTOM_DVE_SPECS[name] = spec
    return op


OP_SQ2 = _register_op("CANNY_SQ2", _Spec(
    body=_sq(_S0) + _sq(_S1),
    reference=lambda in0, in1, s0, s1, imm2:
        ((in0 * in0) + (in1 * in1)).astype(_np.float32)))

OP_WEDGEM = _register_op("CANNY_WEDGEM", _Spec(
    body=((_S0 * _C0 + _S1) * _C1) > _Z,
    reference=lambda in0, in1, s0, s1, imm2:
        (((in0 * _np.float32(s0) + in1) * _np.float32(s1)) > 0)
        .astype(_np.float32)))

OP_IDXF = _register_op("CANNY_IDXF", _Spec(
    body=_sel(_S1 > _Z, _C0 - _S0, _S0),
    reference=lambda in0, in1, s0, s1, imm2:
        _np.where(in1 > 0, _np.float32(s0) - in0, in0).astype(_np.float32)))

OP_V4 = _register_op("CANNY_V4", _Spec(
    body=_S0 - _C0 * (_S0 >= _C1),
    reference=lambda in0, in1, s0, s1, imm2:
        (in0 - _np.float32(s0) * (in0 >= _np.float32(s1))).astype(_np.float32)))

OP_NMSTHIN = _register_op("CANNY_NMSTHIN", _Spec(
    body=_sel(_S1 < _S0, _S0, _Z),
    reference=lambda in0, in1, s0, s1, imm2:
        _np.where(in1 < in0, in0, 0.0).astype(_np.float32)))

OP_PMMASK = _register_op("CANNY_PMMASK", _Spec(
    body=_maxx(_S0 * _C0, _S1 * _C1),
    reference=lambda in0, in1, s0, s1, imm2:
        _np.maximum(in0 * _np.asarray(s0, _np.float32),
                    in1 * _np.asarray(s1, _np.float32)).astype(_np.float32)))


OP_CMPABS = _register_op("CANNY_CMPABS", _Spec(
    # (|Src0|*C0 < |Src1|) -> 1.0/0.0
    body=(_maxx(_S0, _Z - _S0) * _C0) < _maxx(_S1, _Z - _S1),
    reference=lambda in0, in1, s0, s1, imm2:
        ((_np.abs(in0) * _np.float32(s0)) < _np.abs(in1)).astype(_np.float32)))

OP_DSEL = _register_op("CANNY_DSEL", _Spec(
    # select(Src1 > 0, Src0, C0 - Src0)
    body=_sel(_S1 > _Z, _S0, _C0 - _S0),
    reference=lambda in0, in1, s0, s1, imm2:
        _np.where(in1 > 0, in0, _np.float32(s0) - in0).astype(_np.float32)))

OP_SIGNSEL = _register_op("CANNY_SIGNSEL", _Spec(
    # select(Src1 > 0, C0 + Src0, C0 - Src0)
    body=_sel(_S1 > _Z, _C0 + _S0, _C0 - _S0),
    reference=lambda in0, in1, s0, s1, imm2:
        _np.where(in1 > 0, _np.float32(s0) + in0,
                  _np.float32(s0) - in0).astype(_np.float32)))

F32 = np.float32
DT = mybir.dt.float32
OP = mybir.AluOpType
AF = mybir.ActivationFunctionType


class Geo:
    def __init__(self, H=2048, W=2048, n_cores=8, L=86, frame_offs=(0, 86, 170)):
        self.H, self.W, self.n_cores, self.L = H, W, n_cores, L
        self.rpc = H // n_cores
        self.frame_offs = list(frame_offs)          # relative frame starts
        assert self.frame_offs[-1] + L == self.rpc
        self.K = L + 8                               # A'/img frame rows
        self.K2 = L + 2                              # t1/t2/mag frame rows
        self.WP = W + 8                              # padded slab width
        self.n_frames = len(self.frame_offs)


# ----------------------------------------------------------------------------
# host-side prep (bands, masks, slabs)
# ----------------------------------------------------------------------------
def _make_bands(geo, g5, smooth, diff, frames):
    """Bands [K, M] f32: blur [K,L], t1/t2 [K,K2] per frame.
    A' partition k = global row o-4+k;  A' = A/g0 so fold g0 here.
    t1 = smooth_v(blur zero-padded at [0,H)), t2 = diff_v(same)."""
    g5p = np.asarray(g5, np.float64) * float(g5[0])
    H, L, K, K2 = geo.H, geo.L, geo.K, geo.K2
    out = []
    for o in frames:
        def blur_col(r):
            col = np.zeros(K)
            for i in range(5):
                k = (r + i - 2) - (o - 4)
                if 0 <= k < K:
                    col[k] = g5p[i]
            return col
        bb = np.zeros((K, L)); b1 = np.zeros((K, K2)); b2 = np.zeros((K, K2))
        for m in range(L):
            bb[:, m] = blur_col(o + m)
        for m in range(K2):
            r = o - 1 + m
            for j in range(3):
                rb = r + j - 1
                if 0 <= rb < H:
                    b1[:, m] += smooth[j] * blur_col(rb)
                    b2[:, m] += diff[j] * blur_col(rb)
        out.append((bb.astype(F32), b1.astype(F32), b2.astype(F32)))
    return out


def _make_rowmasks(geo, frames):
    """[K2, 2*n_frames]: col f*2+0 = mask_dn, f*2+1 = mask_up."""
    m = np.ones((geo.K2, 2 * len(frames)), F32)
    for fi, o in enumerate(frames):
        for p in range(geo.K2):
            if not (0 <= (o - 2 + p) < geo.H):
                m[p, 2 * fi + 0] = 0.0
            if not (0 <= (o + p) < geo.H):
                m[p, 2 * fi + 1] = 0.0
    return m


def _wedge_consts():
    out = []
    for k in range(4):
        b = (45.0 * k - 157.5) * 3.14159 / 180.0
        out.append((float(F32(-np.sin(b) / np.cos(b))), bool(np.cos(b) > 0)))
    return out


# ----------------------------------------------------------------------------
# the Bass program (SPMD; all boundary behaviour carried by input data)
# ----------------------------------------------------------------------------
def build_program(geo, g1r, g2r):
    nc = bacc.Bacc("TRN2", target_bir_lowering=False, debug=False)
    K, K2, L, W, WP, NF = geo.K, geo.K2, geo.L, geo.W, geo.WP, geo.n_frames
    HW = W // 2                                      # half width
    NQ = HW // 512 if HW >= 512 else 0               # 512-chunks per half

    slab = nc.declare_dram_parameter("slab", [3, geo.rpc + 8, WP], DT, isOutput=False)
    bandb_d = nc.declare_dram_parameter("bandb", [NF, K, L], DT, isOutput=False)
    band1_d = nc.declare_dram_parameter("band1", [NF, K, K2], DT, isOutput=False)
    band2_d = nc.declare_dram_parameter("band2", [NF, K, K2], DT, isOutput=False)
    rmask_d = nc.declare_dram_parameter("rmask", [K2, 2 * NF], DT, isOutput=False)
    thr_d = nc.declare_dram_parameter("thr", [K2, 1], DT, isOutput=False)

    blur_o = nc.declare_dram_parameter("blurred", [3, geo.rpc, W], DT, isOutput=True)
    grad_o = nc.declare_dram_parameter("grad", [geo.rpc, W], DT, isOutput=True)
    orient_o = nc.declare_dram_parameter("orient", [geo.rpc, W], DT, isOutput=True)
    thin_o = nc.declare_dram_parameter("thin", [geo.rpc, W], DT, isOutput=True)
    thresh_o = nc.declare_dram_parameter("thresh", [geo.rpc, W], DT, isOutput=True)
    early_o = nc.declare_dram_parameter("early", [geo.rpc, W], DT, isOutput=True)

    wedge = _wedge_consts()
    wedge_t1 = abs(wedge[0][0])
    wedge_t2 = abs(wedge[1][0])

    def psum_conv(pool, bnd, A, rows, half):
        """One half-width vertical band conv on PE into a PSUM tile."""
        pt = pool.tile([rows, HW], DT, tag="pt")
        cs = half * HW
        if NQ:
            for q in range(NQ):
                nc.tensor.matmul(pt[:, q * 512:(q + 1) * 512], bnd,
                                 A[:, cs + q * 512: cs + (q + 1) * 512],
                                 start=True, stop=True)
        else:
            nc.tensor.matmul(pt[:], bnd, A[:, cs:cs + HW], start=True, stop=True)
        return pt

    from contextlib import ExitStack
    with tile.TileContext(nc) as tc, ExitStack() as ctx:
        consts = ctx.enter_context(tc.tile_pool(name="consts", bufs=1))
        img_p = ctx.enter_context(tc.tile_pool(name="img", bufs=2))
        a_p = ctx.enter_context(tc.tile_pool(name="A", bufs=2))
        tmp_p = ctx.enter_context(tc.tile_pool(name="tmp", bufs=3))
        t12_p = ctx.enter_context(tc.tile_pool(name="t12", bufs=2))
        gxy_p = ctx.enter_context(tc.tile_pool(name="gxy", bufs=1))
        acc_p = ctx.enter_context(tc.tile_pool(name="acc", bufs=1))
        idx_p = ctx.enter_context(tc.tile_pool(name="idx", bufs=1))
        shift_p = ctx.enter_context(tc.tile_pool(name="shift", bufs=1))
        pm_p = ctx.enter_context(tc.tile_pool(name="pm", bufs=1))
        out_p = ctx.enter_context(tc.tile_pool(name="out", bufs=1))
        outp2_p = ctx.enter_context(tc.tile_pool(name="out2", bufs=2))
        pb_p = ctx.enter_context(tc.tile_pool(name="pb", bufs=2, space="PSUM"))
        pt_p = ctx.enter_context(tc.tile_pool(name="pt", bufs=2, space="PSUM"))

        bandb = consts.tile([K, NF * L], DT)
        band1 = consts.tile([K, NF * K2], DT)
        band2 = consts.tile([K, NF * K2], DT)
        rmask = consts.tile([K2, 2 * NF], DT)
        thr = consts.tile([K2, 1], DT)
        for f in range(NF):
            nc.sync.dma_start(bandb[:, f * L:(f + 1) * L], bandb_d[f])
            nc.sync.dma_start(band1[:, f * K2:(f + 1) * K2], band1_d[f])
            nc.sync.dma_start(band2[:, f * K2:(f + 1) * K2], band2_d[f])
        nc.sync.dma_start(rmask[:], rmask_d[:])
        nc.sync.dma_start(thr[:], thr_d[:])

        for f, o in enumerate(geo.frame_offs):
            bb = bandb[:, f * L:(f + 1) * L]
            b1 = band1[:, f * K2:(f + 1) * K2]
            b2 = band2[:, f * K2:(f + 1) * K2]
            mag = acc_p.tile([K2, W + 4], DT, tag="mag")
            gxs = acc_p.tile([K2, W], DT, tag="gxs")
            gys = acc_p.tile([K2, W], DT, tag="gys")
            nc.gpsimd.memset(mag[:, 0:2], 0.0)
            nc.gpsimd.memset(mag[:, W + 2:W + 4], 0.0)

            for c in range(3):
                x = img_p.tile([K, WP], DT, tag="x")
                nc.sync.dma_start(x[:], slab[c, o:o + K, :])

                # horizontal 5-tap blur (w computed in place over u)
                u = tmp_p.tile([K, W], DT, tag="tmp")
                v = tmp_p.tile([K, W], DT, tag="tmp")
                nc.vector.tensor_tensor(u[:], x[:, 3:3 + W], x[:, 5:5 + W], OP.add)
                nc.vector.tensor_tensor(v[:], x[:, 2:2 + W], x[:, 6:6 + W], OP.add)
                nc.vector.scalar_tensor_tensor(u[:], u[:], g1r, v[:], OP.mult, OP.add)
                A = a_p.tile([K, W], DT, tag="A")
                nc.vector.scalar_tensor_tensor(A[:], x[:, 4:4 + W], g2r, u[:],
                                               OP.mult, OP.add)

                # vertical band convs on PE
                blursb = out_p.tile([L, W], DT, tag="blur")
                for half in range(2):
                    pb = psum_conv(pb_p, bb, A, L, half)
                    nc.scalar.copy(blursb[:, half * HW:(half + 1) * HW], pb[:])
                nc.sync.dma_start(blur_o[c, o:o + L, :], blursb[:])
                t1 = t12_p.tile([K2, W + 4], DT, tag="t12")
                t2 = t12_p.tile([K2, W + 4], DT, tag="t12")
                for t_sb, bnd in ((t1, b1), (t2, b2)):
                    nc.gpsimd.memset(t_sb[:, 0:2], 0.0)
                    nc.gpsimd.memset(t_sb[:, W + 2:W + 4], 0.0)
                    for half in range(2):
                        pt = psum_conv(pt_p, bnd, A, K2, half)
                        nc.scalar.copy(t_sb[:, 2 + half * HW:2 + (half + 1) * HW],
                                       pt[:])

                # sobel horizontal parts (c0 writes gxs/gys directly)
                gx = gxs if c == 0 else gxy_p.tile([K2, W], DT, tag="gx")
                nc.vector.scalar_tensor_tensor(gx[:], t1[:, 3:3 + W], -1.0,
                                               t1[:, 1:1 + W], OP.mult, OP.add)
                tt = tmp_p.tile([K2, W], DT, tag="tmp")
                nc.vector.tensor_tensor(tt[:], t2[:, 1:1 + W], t2[:, 3:3 + W], OP.add)
                gy = gys if c == 0 else gxy_p.tile([K2, W], DT, tag="gy")
                nc.vector.scalar_tensor_tensor(gy[:], t2[:, 2:2 + W], 2.0, tt[:],
                                               OP.mult, OP.add)

                # magnitude: msq = gx^2 + gy^2; sqrt in place for c>0
                msq = tmp_p.tile([K2, W], DT, tag="tmp")
                nc.vector._custom_dve(OP_SQ2, out=msq[:], in0=gx[:], in1=gy[:])
                if c == 0:
                    nc.scalar.activation(mag[:, 2:2 + W], msq[:], AF.Sqrt)
                else:
                    nc.scalar.activation(msq[:], msq[:], AF.Sqrt)
                    nc.vector.tensor_tensor(mag[:, 2:2 + W], mag[:, 2:2 + W],
                                            msq[:], OP.add)
                    nc.vector.tensor_tensor(gxs[:], gxs[:], gx[:], OP.add)
                    nc.vector.tensor_tensor(gys[:], gys[:], gy[:], OP.add)

            # ---- orientation sector: |gxs|*t <> |gys| folds + sign mapping ----
            s0t = shift_p.tile([K2, W], DT, tag="mu")
            nc.vector._custom_dve(OP_CMPABS, out=s0t[:], in0=gxs[:], in1=gys[:],
                                  s0=wedge_t1)
            s1t = shift_p.tile([K2, W], DT, tag="md")
            nc.vector._custom_dve(OP_CMPABS, out=s1t[:], in0=gxs[:], in1=gys[:],
                                  s0=wedge_t2)
            bt = tmp_p.tile([K2, W], DT, tag="tmp")
            nc.vector.tensor_tensor(bt[:], s0t[:], s1t[:], OP.add)
            dd = idx_p.tile([K2, W], DT, tag="S")
            nc.vector._custom_dve(OP_DSEL, out=dd[:], in0=bt[:], in1=gxs[:], s0=4.0)
            idxf = idx_p.tile([K2, W], DT, tag="idxf")
            nc.vector._custom_dve(OP_SIGNSEL, out=idxf[:], in0=dd[:], in1=gys[:],
                                  s0=4.0)
            orient_t = out_p.tile([K2, W], DT, tag="orient")
            nc.scalar.mul(orient_t[:], idxf[:], 45.0)
            nc.sync.dma_start(orient_o[o:o + L, :], orient_t[1:1 + L, :])

            # ---- NMS ----
            magup = shift_p.tile([K2, W + 4], DT, tag="mu")
            magdn = shift_p.tile([K2, W + 4], DT, tag="md")
            nc.sync.dma_start(magup[0:K2 - 1, :], mag[1:K2, :])
            nc.sync.dma_start(magup[K2 - 1:K2, :], mag[K2 - 1:K2, :])
            nc.sync.dma_start(magdn[1:K2, :], mag[0:K2 - 1, :])
            nc.sync.dma_start(magdn[0:1, :], mag[0:1, :])

            V4 = idx_p.tile([K2, W], DT, tag="S")
            nc.vector._custom_dve(OP_V4, out=V4[:], in0=idxf[:], s0=4.0, s1=3.5)
            sel = gxy_p.tile([K2, W], DT, tag="gy")
            nc.vector.tensor_tensor(sel[:], mag[:, 1:1 + W], mag[:, 3:3 + W], OP.max)
            mu_ap = rmask[:, 2 * f + 1:2 * f + 2]
            md_ap = rmask[:, 2 * f:2 * f + 1]
            for j, ta, ca, tb, cb in [(1, magup, 3, magdn, 1),
                                      (2, magup, 2, magdn, 2),
                                      (3, magup, 1, magdn, 3)]:
                pmj = pm_p.tile([K2, W], DT, tag="pm")
                nc.vector._custom_dve(OP_PMMASK, out=pmj[:],
                                      in0=ta[:, ca:ca + W], in1=tb[:, cb:cb + W],
                                      s0=mu_ap, s1=md_ap)
                mj = tmp_p.tile([K2, W], DT, tag="tmp")
                nc.vector.tensor_scalar(mj[:], V4[:], float(j), None, OP.is_equal)
                nc.vector.copy_predicated(sel[:], mj[:].bitcast(mybir.dt.int32),
                                          pmj[:])

            magc = mag[:, 2:2 + W]
            thin = outp2_p.tile([K2, W], DT, tag="o")
            nc.vector._custom_dve(OP_NMSTHIN, out=thin[:], in0=magc, in1=sel[:])
            thresh = outp2_p.tile([K2, W], DT, tag="o")
            nc.vector.scalar_tensor_tensor(thresh[:], thin[:], thr[:, 0:1], thin[:],
                                           OP.is_ge, OP.mult)
            early = outp2_p.tile([K2, W], DT, tag="o")
            nc.vector.scalar_tensor_tensor(early[:], magc, thr[:, 0:1], magc,
                                           OP.is_ge, OP.mult)
            nc.sync.dma_start(grad_o[o:o + L, :], mag[1:1 + L, 2:2 + W])
            nc.sync.dma_start(thin_o[o:o + L, :], thin[1:1 + L, :])
            nc.sync.dma_start(thresh_o[o:o + L, :], thresh[1:1 + L, :])
            nc.sync.dma_start(early_o[o:o + L, :], early[1:1 + L, :])
    nc.finalize()
    return nc


_prog_cache = {}


def _get_program(geo, g1r, g2r):
    key = (geo.H, geo.W, geo.n_cores, geo.L, tuple(geo.frame_offs),
           float(g1r), float(g2r))
    if key not in _prog_cache:
        _prog_cache[key] = build_program(geo, g1r, g2r)
    return _prog_cache[key]


def make_in_maps(geo, img, thr_val, g5, smooth, diff):
    H, W, rpc = geo.H, geo.W, geo.rpc
    padded = np.zeros((3, H + 8, W + 8), F32)
    padded[:, 4:H + 4, 4:W + 4] = img[0]
    in_maps = []
    for core in range(geo.n_cores):
        s = core * rpc
        frames = [s + fo for fo in geo.frame_offs]
        bands = _make_bands(geo, g5, smooth, diff, frames)
        in_maps.append(dict(
            slab=np.ascontiguousarray(padded[:, s:s + rpc + 8, :]),
            bandb=np.stack([b[0] for b in bands]),
            band1=np.stack([b[1] for b in bands]),
            band2=np.stack([b[2] for b in bands]),
            rmask=_make_rowmasks(geo, frames),
            thr=np.full((geo.K2, 1), thr_val, F32),
        ))
    return in_maps


def assemble(geo, outs):
    H, W, rpc = geo.H, geo.W, geo.rpc
    blurred = np.zeros((1, 3, H, W), F32)
    planes = [np.zeros((1, 1, H, W), F32) for _ in range(5)]
    names = ["grad", "orient", "thin", "thresh", "early"]
    for core in range(geo.n_cores):
        s = core * rpc
        r = outs[core]
        blurred[0, :, s:s + rpc] = r["blurred"]
        for p, n in zip(planes, names):
            p[0, 0, s:s + rpc] = r[n]
    return (blurred, planes[0], planes[1], planes[2], planes[3], planes[4])


def kernel(img, threshold, gauss_h, gauss_v, sobel_h, sobel_v, dir_w):
    geo = Geo()
    img = np.asarray(img, F32)
    thr_val = float(np.asarray(threshold, F32).reshape(-1)[0])
    g5 = np.asarray(gauss_h, F32).reshape(-1)
    sob = np.asarray(sobel_h, F32).reshape(3, 3)
    diff = [float(sob[0, 0]), float(sob[0, 1]), float(sob[0, 2])]
    smooth = [float(sob[0, 0] / sob[0, 0]), float(sob[1, 0] / sob[0, 0]),
              float(sob[2, 0] / sob[0, 0])]
    g1r, g2r = float(g5[1] / g5[0]), float(g5[2] / g5[0])

    nc = _get_program(geo, g1r, g2r)
    in_maps = make_in_maps(geo, img, thr_val, g5, smooth, diff)
    global LAST_RESULTS
    kw = dict(TRACE_KWARGS) if TRACE else {}
    res = run_bass_kernel_spmd(nc, in_maps, core_ids=list(range(geo.n_cores)),
                               trace=TRACE, **kw)
    LAST_RESULTS = res
    return assemble(geo, res.results)


# test-harness hooks (the grader just calls kernel(**inputs))
TRACE = False
TRACE_KWARGS = {}
LAST_RESULTS = None


# revision 28
# speedup vs baseline: 1.0526x; 1.0526x over previous
"""CannyNet Trainium2 Bass kernel.

kernel(**inputs) takes the complete unsharded inputs (img [1,3,2048,2048],
threshold [1], gauss/sobel/dir filter tensors) and returns the reference
tuple (blurred_img, grad_mag, orient, thin_edges, thresholded,
early_threshold) computed on 8 NeuronCores.

Sharding: H is split spatially across the 8 cores (256 rows each).  Each
core receives a zero-padded slab of its rows (+4 halo rows, +4 halo cols)
and processes three 86-row "frames" that are fully self-contained: the
vertical convolutions are band matmuls on the tensor engine (the band
matrices encode the frame offset, the gaussian/sobel taps, and the exact
zero-padding behaviour of the reference at the global image edges), while
horizontal convolutions and pointwise math run on the vector, scalar and
gpsimd engines.  The per-pixel orientation sector (for NMS and the orient
output) is computed with sign tests against the 8 reference sector
boundaries instead of atan2.
"""

import numpy as np

try:  # the grading env has concourse importable; fall back to container path
    import concourse.bass as _b  # noqa: F401
except ImportError:  # pragma: no cover
    import sys
    for _p in ("/opt/trn_rl_repo",):
        if _p not in sys.path:
            sys.path.insert(0, _p)

import concourse.bass as bass
import concourse.bacc as bacc
import concourse.mybir as mybir
import concourse.tile as tile
from concourse.bass_utils import run_bass_kernel_spmd


# ----------------------------------------------------------------------------
# custom fused DVE ops (registered into the per-NEFF DVE table)
# ----------------------------------------------------------------------------
from concourse import dve_ops as _dve_ops
from concourse.dve_spec import (Spec as _Spec, Src0 as _S0, Src1 as _S1,
                                C0 as _C0, C1 as _C1, Zero as _Z,
                                select as _sel, sq as _sq, maxx as _maxx,
                                lower as _dve_lower, _has_src1 as _has_src1)
from concourse.dve_uop import DveOpSpec as _DveOpSpec
import numpy as _np


def _register_op(name, spec, subdim=False):
    for op in _dve_ops.OPS:
        if op.name == name:
            return op
    opcode = _dve_ops._CUSTOM_DVE_ROW_BASE + len(_dve_ops.OPS)
    assert opcode < 0x20, "custom DVE op row overflow"
    shas = {}
    for ver in ("v3", "v4"):
        s = _DveOpSpec(name=name, opcode=opcode, uops=_dve_lower(spec, ver=ver),
                       rd1_en=_has_src1(spec))
        shas[ver] = s.sha(ver)
    op = _dve_ops.DveOp(name, spec, subdim, shas)
    _dve_ops.OPS.append(op)
    _dve_ops._SUB_OPCODE_FOR_NAME[name] = opcode
    _dve_ops.CUSTOM_DVE_SPECS[name] = spec
    return op


OP_SQ2 = _register_op("CANNY_SQ2", _Spec(
    body=_sq(_S0) + _sq(_S1),
    reference=lambda in0, in1, s0, s1, imm2:
        ((in0 * in0) + (in1 * in1)).astype(_np.float32)))

OP_WEDGEM = _register_op("CANNY_WEDGEM", _Spec(
    body=((_S0 * _C0 + _S1) * _C1) > _Z,
    reference=lambda in0, in1, s0, s1, imm2:
        (((in0 * _np.float32(s0) + in1) * _np.float32(s1)) > 0)
        .astype(_np.float32)))

OP_IDXF = _register_op("CANNY_IDXF", _Spec(
    body=_sel(_S1 > _Z, _C0 - _S0, _S0),
    reference=lambda in0, in1, s0, s1, imm2:
        _np.where(in1 > 0, _np.float32(s0) - in0, in0).astype(_np.float32)))

OP_V4 = _register_op("CANNY_V4", _Spec(
    body=_S0 - _C0 * (_S0 >= _C1),
    reference=lambda in0, in1, s0, s1, imm2:
        (in0 - _np.float32(s0) * (in0 >= _np.float32(s1))).astype(_np.float32)))

OP_NMSTHIN = _register_op("CANNY_NMSTHIN", _Spec(
    body=_sel(_S1 < _S0, _S0, _Z),
    reference=lambda in0, in1, s0, s1, imm2:
        _np.where(in1 < in0, in0, 0.0).astype(_np.float32)))

OP_PMMASK = _register_op("CANNY_PMMASK", _Spec(
    body=_maxx(_S0 * _C0, _S1 * _C1),
    reference=lambda in0, in1, s0, s1, imm2:
        _np.maximum(in0 * _np.asarray(s0, _np.float32),
                    in1 * _np.asarray(s1, _np.float32)).astype(_np.float32)))


OP_CMPABS = _register_op("CANNY_CMPABS", _Spec(
    # (|Src0|*C0 < |Src1|) -> 1.0/0.0
    body=(_maxx(_S0, _Z - _S0) * _C0) < _maxx(_S1, _Z - _S1),
    reference=lambda in0, in1, s0, s1, imm2:
        ((_np.abs(in0) * _np.float32(s0)) < _np.abs(in1)).astype(_np.float32)))

OP_DSEL = _register_op("CANNY_DSEL", _Spec(
    # select(Src1 > 0, Src0, C0 - Src0)
    body=_sel(_S1 > _Z, _S0, _C0 - _S0),
    reference=lambda in0, in1, s0, s1, imm2:
        _np.where(in1 > 0, in0, _np.float32(s0) - in0).astype(_np.float32)))

OP_SIGNSEL = _register_op("CANNY_SIGNSEL", _Spec(
    # select(Src1 > 0, C0 + Src0, C0 - Src0)
    body=_sel(_S1 > _Z, _C0 + _S0, _C0 - _S0),
    reference=lambda in0, in1, s0, s1, imm2:
        _np.where(in1 > 0, _np.float32(s0) + in0,
                  _np.float32(s0) - in0).astype(_np.float32)))

F32 = np.float32
DT = mybir.dt.float32
OP = mybir.AluOpType
AF = mybir.ActivationFunctionType


class Geo:
    def __init__(self, H=2048, W=2048, n_cores=8, L=86, frame_offs=(0, 86, 170)):
        self.H, self.W, self.n_cores, self.L = H, W, n_cores, L
        self.rpc = H // n_cores
        self.frame_offs = list(frame_offs)          # relative frame starts
        assert self.frame_offs[-1] + L == self.rpc
        self.K = L + 8                               # A'/img frame rows
        self.K2 = L + 2                              # t1/t2/mag frame rows
        self.WP = W + 8                              # padded slab width
        self.n_frames = len(self.frame_offs)


# ----------------------------------------------------------------------------
# host-side prep (bands, masks, slabs)
# ----------------------------------------------------------------------------
def _make_bands(geo, g5, smooth, diff, frames):
    """Bands [K, M] f32: blur [K,L], t1/t2 [K,K2] per frame.
    A' partition k = global row o-4+k;  A' = A/g0 so fold g0 here.
    t1 = smooth_v(blur zero-padded at [0,H)), t2 = diff_v(same)."""
    g5p = np.asarray(g5, np.float64) * float(g5[0])
    H, L, K, K2 = geo.H, geo.L, geo.K, geo.K2
    out = []
    for o in frames:
        def blur_col(r):
            col = np.zeros(K)
            for i in range(5):
                k = (r + i - 2) - (o - 4)
                if 0 <= k < K:
                    col[k] = g5p[i]
            return col
        bb = np.zeros((K, L)); b1 = np.zeros((K, K2)); b2 = np.zeros((K, K2))
        for m in range(L):
            bb[:, m] = blur_col(o + m)
        for m in range(K2):
            r = o - 1 + m
            for j in range(3):
                rb = r + j - 1
                if 0 <= rb < H:
                    b1[:, m] += smooth[j] * blur_col(rb)
                    b2[:, m] += diff[j] * blur_col(rb)
        out.append((bb.astype(F32), b1.astype(F32), b2.astype(F32)))
    return out


def _make_rowmasks(geo, frames):
    """[K2, 2*n_frames]: col f*2+0 = mask_dn, f*2+1 = mask_up."""
    m = np.ones((geo.K2, 2 * len(frames)), F32)
    for fi, o in enumerate(frames):
        for p in range(geo.K2):
            if not (0 <= (o - 2 + p) < geo.H):
                m[p, 2 * fi + 0] = 0.0
            if not (0 <= (o + p) < geo.H):
                m[p, 2 * fi + 1] = 0.0
    return m


def _wedge_consts():
    out = []
    for k in range(4):
        b = (45.0 * k - 157.5) * 3.14159 / 180.0
        out.append((float(F32(-np.sin(b) / np.cos(b))), bool(np.cos(b) > 0)))
    return out


# ----------------------------------------------------------------------------
# the Bass program (SPMD; all boundary behaviour carried by input data)
# ----------------------------------------------------------------------------
def build_program(geo, g1r, g2r):
    nc = bacc.Bacc("TRN2", target_bir_lowering=False, debug=False)
    K, K2, L, W, WP, NF = geo.K, geo.K2, geo.L, geo.W, geo.WP, geo.n_frames
    HW = W // 2                                      # half width
    NQ = HW // 512 if HW >= 512 else 0               # 512-chunks per half

    slab = nc.declare_dram_parameter("slab", [3, geo.rpc + 8, WP], DT, isOutput=False)
    bandb_d = nc.declare_dram_parameter("bandb", [NF, K, L], DT, isOutput=False)
    band1_d = nc.declare_dram_parameter("band1", [NF, K, K2], DT, isOutput=False)
    band2_d = nc.declare_dram_parameter("band2", [NF, K, K2], DT, isOutput=False)
    rmask_d = nc.declare_dram_parameter("rmask", [K2, 2 * NF], DT, isOutput=False)
    thr_d = nc.declare_dram_parameter("thr", [K2, 1], DT, isOutput=False)

    blur_o = nc.declare_dram_parameter("blurred", [3, geo.rpc, W], DT, isOutput=True)
    grad_o = nc.declare_dram_parameter("grad", [geo.rpc, W], DT, isOutput=True)
    orient_o = nc.declare_dram_parameter("orient", [geo.rpc, W], DT, isOutput=True)
    thin_o = nc.declare_dram_parameter("thin", [geo.rpc, W], DT, isOutput=True)
    thresh_o = nc.declare_dram_parameter("thresh", [geo.rpc, W], DT, isOutput=True)
    early_o = nc.declare_dram_parameter("early", [geo.rpc, W], DT, isOutput=True)

    wedge = _wedge_consts()
    wedge_t1 = abs(wedge[0][0])
    wedge_t2 = abs(wedge[1][0])

    def psum_conv(pool, bnd, A, rows, half):
        """One half-width vertical band conv on PE into a PSUM tile."""
        pt = pool.tile([rows, HW], DT, tag="pt")
        cs = half * HW
        if NQ:
            for q in range(NQ):
                nc.tensor.matmul(pt[:, q * 512:(q + 1) * 512], bnd,
                                 A[:, cs + q * 512: cs + (q + 1) * 512],
                                 start=True, stop=True)
        else:
            nc.tensor.matmul(pt[:], bnd, A[:, cs:cs + HW], start=True, stop=True)
        return pt

    from contextlib import ExitStack
    with tile.TileContext(nc) as tc, ExitStack() as ctx:
        consts = ctx.enter_context(tc.tile_pool(name="consts", bufs=1))
        img_p = ctx.enter_context(tc.tile_pool(name="img", bufs=2))
        a_p = ctx.enter_context(tc.tile_pool(name="A", bufs=2))
        tmp_p = ctx.enter_context(tc.tile_pool(name="tmp", bufs=3))
        t12_p = ctx.enter_context(tc.tile_pool(name="t12", bufs=2))
        gxy_p = ctx.enter_context(tc.tile_pool(name="gxy", bufs=1))
        acc_p = ctx.enter_context(tc.tile_pool(name="acc", bufs=1))
        idx_p = ctx.enter_context(tc.tile_pool(name="idx", bufs=1))
        shift_p = ctx.enter_context(tc.tile_pool(name="shift", bufs=1))
        pm_p = ctx.enter_context(tc.tile_pool(name="pm", bufs=1))
        out_p = ctx.enter_context(tc.tile_pool(name="out", bufs=1))
        outp2_p = ctx.enter_context(tc.tile_pool(name="out2", bufs=2))
        pb_p = ctx.enter_context(tc.tile_pool(name="pb", bufs=2, space="PSUM"))
        pt_p = ctx.enter_context(tc.tile_pool(name="pt", bufs=2, space="PSUM"))

        bandb = consts.tile([K, NF * L], DT)
        band1 = consts.tile([K, NF * K2], DT)
        band2 = consts.tile([K, NF * K2], DT)
        rmask = consts.tile([K2, 2 * NF], DT)
        thr = consts.tile([K2, 1], DT)
        for f in range(NF):
            nc.sync.dma_start(bandb[:, f * L:(f + 1) * L], bandb_d[f])
            nc.sync.dma_start(band1[:, f * K2:(f + 1) * K2], band1_d[f])
            nc.sync.dma_start(band2[:, f * K2:(f + 1) * K2], band2_d[f])
        nc.sync.dma_start(rmask[:], rmask_d[:])
        nc.sync.dma_start(thr[:], thr_d[:])

        for f, o in enumerate(geo.frame_offs):
            bb = bandb[:, f * L:(f + 1) * L]
            b1 = band1[:, f * K2:(f + 1) * K2]
            b2 = band2[:, f * K2:(f + 1) * K2]
            mag = acc_p.tile([K2, W + 4], DT, tag="mag")
            gxs = acc_p.tile([K2, W], DT, tag="gxs")
            gys = acc_p.tile([K2, W], DT, tag="gys")
            nc.gpsimd.memset(mag[:, 0:2], 0.0)
            nc.gpsimd.memset(mag[:, W + 2:W + 4], 0.0)

            for c in range(3):
                x = img_p.tile([K, WP], DT, tag="x")
                nc.sync.dma_start(x[:], slab[c, o:o + K, :])

                # horizontal 5-tap blur (w computed in place over u)
                u = tmp_p.tile([K, W], DT, tag="tmp")
                v = tmp_p.tile([K, W], DT, tag="tmp")
                nc.vector.tensor_tensor(u[:], x[:, 3:3 + W], x[:, 5:5 + W], OP.add)
                nc.vector.tensor_tensor(v[:], x[:, 2:2 + W], x[:, 6:6 + W], OP.add)
                nc.vector.scalar_tensor_tensor(u[:], u[:], g1r, v[:], OP.mult, OP.add)
                A = a_p.tile([K, W], DT, tag="A")
                nc.vector.scalar_tensor_tensor(A[:], x[:, 4:4 + W], g2r, u[:],
                                               OP.mult, OP.add)

                # vertical band convs on PE
                blursb = outp2_p.tile([L, W], DT, tag="blur")
                for half in range(2):
                    pb = psum_conv(pb_p, bb, A, L, half)
                    nc.scalar.copy(blursb[:, half * HW:(half + 1) * HW], pb[:])
                nc.sync.dma_start(blur_o[c, o:o + L, :], blursb[:])
                t1 = t12_p.tile([K2, W + 4], DT, tag="t12")
                t2 = t12_p.tile([K2, W + 4], DT, tag="t12")
                for t_sb, bnd in ((t1, b1), (t2, b2)):
                    nc.gpsimd.memset(t_sb[:, 0:2], 0.0)
                    nc.gpsimd.memset(t_sb[:, W + 2:W + 4], 0.0)
                    for half in range(2):
                        pt = psum_conv(pt_p, bnd, A, K2, half)
                        nc.scalar.copy(t_sb[:, 2 + half * HW:2 + (half + 1) * HW],
                                       pt[:])

                # sobel horizontal parts (c0 writes gxs/gys directly)
                gx = gxs if c == 0 else gxy_p.tile([K2, W], DT, tag="gx")
                nc.vector.scalar_tensor_tensor(gx[:], t1[:, 3:3 + W], -1.0,
                                               t1[:, 1:1 + W], OP.mult, OP.add)
                tt = tmp_p.tile([K2, W], DT, tag="tmp")
                nc.vector.tensor_tensor(tt[:], t2[:, 1:1 + W], t2[:, 3:3 + W], OP.add)
                gy = gys if c == 0 else gxy_p.tile([K2, W], DT, tag="gy")
                nc.vector.scalar_tensor_tensor(gy[:], t2[:, 2:2 + W], 2.0, tt[:],
                                               OP.mult, OP.add)

                # magnitude: msq = gx^2 + gy^2; sqrt in place for c>0
                msq = tmp_p.tile([K2, W], DT, tag="tmp")
                nc.vector._custom_dve(OP_SQ2, out=msq[:], in0=gx[:], in1=gy[:])
                if c == 0:
                    nc.scalar.activation(mag[:, 2:2 + W], msq[:], AF.Sqrt)
                else:
                    nc.scalar.activation(msq[:], msq[:], AF.Sqrt)
                    nc.vector.tensor_tensor(mag[:, 2:2 + W], mag[:, 2:2 + W],
                                            msq[:], OP.add)
                    nc.vector.tensor_tensor(gxs[:], gxs[:], gx[:], OP.add)
                    nc.vector.tensor_tensor(gys[:], gys[:], gy[:], OP.add)

            # ---- orientation sector: |gxs|*t <> |gys| folds + sign mapping ----
            s0t = shift_p.tile([K2, W], DT, tag="mu")
            nc.vector._custom_dve(OP_CMPABS, out=s0t[:], in0=gxs[:], in1=gys[:],
                                  s0=wedge_t1)
            s1t = shift_p.tile([K2, W], DT, tag="md")
            nc.vector._custom_dve(OP_CMPABS, out=s1t[:], in0=gxs[:], in1=gys[:],
                                  s0=wedge_t2)
            bt = tmp_p.tile([K2, W], DT, tag="tmp")
            nc.vector.tensor_tensor(bt[:], s0t[:], s1t[:], OP.add)
            dd = idx_p.tile([K2, W], DT, tag="S")
            nc.vector._custom_dve(OP_DSEL, out=dd[:], in0=bt[:], in1=gxs[:], s0=4.0)
            idxf = idx_p.tile([K2, W], DT, tag="idxf")
            nc.vector._custom_dve(OP_SIGNSEL, out=idxf[:], in0=dd[:], in1=gys[:],
                                  s0=4.0)
            orient_t = out_p.tile([K2, W], DT, tag="orient")
            nc.scalar.mul(orient_t[:], idxf[:], 45.0)
            nc.sync.dma_start(orient_o[o:o + L, :], orient_t[1:1 + L, :])

            # ---- NMS ----
            magup = shift_p.tile([K2, W + 4], DT, tag="mu")
            magdn = shift_p.tile([K2, W + 4], DT, tag="md")
            nc.sync.dma_start(magup[0:K2 - 1, :], mag[1:K2, :])
            nc.sync.dma_start(magup[K2 - 1:K2, :], mag[K2 - 1:K2, :])
            nc.sync.dma_start(magdn[1:K2, :], mag[0:K2 - 1, :])
            nc.sync.dma_start(magdn[0:1, :], mag[0:1, :])

            V4 = idx_p.tile([K2, W], DT, tag="S")
            nc.vector._custom_dve(OP_V4, out=V4[:], in0=idxf[:], s0=4.0, s1=3.5)
            sel = gxy_p.tile([K2, W], DT, tag="gy")
            nc.vector.tensor_tensor(sel[:], mag[:, 1:1 + W], mag[:, 3:3 + W], OP.max)
            mu_ap = rmask[:, 2 * f + 1:2 * f + 2]
            md_ap = rmask[:, 2 * f:2 * f + 1]
            for j, ta, ca, tb, cb in [(1, magup, 3, magdn, 1),
                                      (2, magup, 2, magdn, 2),
                                      (3, magup, 1, magdn, 3)]:
                pmj = pm_p.tile([K2, W], DT, tag="pm")
                nc.vector._custom_dve(OP_PMMASK, out=pmj[:],
                                      in0=ta[:, ca:ca + W], in1=tb[:, cb:cb + W],
                                      s0=mu_ap, s1=md_ap)
                mj = tmp_p.tile([K2, W], DT, tag="tmp")
                nc.vector.tensor_scalar(mj[:], V4[:], float(j), None, OP.is_equal)
                nc.vector.copy_predicated(sel[:], mj[:].bitcast(mybir.dt.int32),
                                          pmj[:])

            magc = mag[:, 2:2 + W]
            thin = outp2_p.tile([K2, W], DT, tag="o")
            nc.vector._custom_dve(OP_NMSTHIN, out=thin[:], in0=magc, in1=sel[:])
            thresh = outp2_p.tile([K2, W], DT, tag="o")
            nc.vector.scalar_tensor_tensor(thresh[:], thin[:], thr[:, 0:1], thin[:],
                                           OP.is_ge, OP.mult)
            early = outp2_p.tile([K2, W], DT, tag="o")
            nc.vector.scalar_tensor_tensor(early[:], magc, thr[:, 0:1], magc,
                                           OP.is_ge, OP.mult)
            nc.sync.dma_start(grad_o[o:o + L, :], mag[1:1 + L, 2:2 + W])
            nc.sync.dma_start(thin_o[o:o + L, :], thin[1:1 + L, :])
            nc.sync.dma_start(thresh_o[o:o + L, :], thresh[1:1 + L, :])
            nc.sync.dma_start(early_o[o:o + L, :], early[1:1 + L, :])
    nc.finalize()
    return nc


_prog_cache = {}


def _get_program(geo, g1r, g2r):
    key = (geo.H, geo.W, geo.n_cores, geo.L, tuple(geo.frame_offs),
           float(g1r), float(g2r))
    if key not in _prog_cache:
        _prog_cache[key] = build_program(geo, g1r, g2r)
    return _prog_cache[key]


def make_in_maps(geo, img, thr_val, g5, smooth, diff):
    H, W, rpc = geo.H, geo.W, geo.rpc
    padded = np.zeros((3, H + 8, W + 8), F32)
    padded[:, 4:H + 4, 4:W + 4] = img[0]
    in_maps = []
    for core in range(geo.n_cores):
        s = core * rpc
        frames = [s + fo for fo in geo.frame_offs]
        bands = _make_bands(geo, g5, smooth, diff, frames)
        in_maps.append(dict(
            slab=np.ascontiguousarray(padded[:, s:s + rpc + 8, :]),
            bandb=np.stack([b[0] for b in bands]),
            band1=np.stack([b[1] for b in bands]),
            band2=np.stack([b[2] for b in bands]),
            rmask=_make_rowmasks(geo, frames),
            thr=np.full((geo.K2, 1), thr_val, F32),
        ))
    return in_maps


def assemble(geo, outs):
    H, W, rpc = geo.H, geo.W, geo.rpc
    blurred = np.zeros((1, 3, H, W), F32)
    planes = [np.zeros((1, 1, H, W), F32) for _ in range(5)]
    names = ["grad", "orient", "thin", "thresh", "early"]
    for core in range(geo.n_cores):
        s = core * rpc
        r = outs[core]
        blurred[0, :, s:s + rpc] = r["blurred"]
        for p, n in zip(planes, names):
            p[0, 0, s:s + rpc] = r[n]
    return (blurred, planes[0], planes[1], planes[2], planes[3], planes[4])


def kernel(img, threshold, gauss_h, gauss_v, sobel_h, sobel_v, dir_w):
    geo = Geo()
    img = np.asarray(img, F32)
    thr_val = float(np.asarray(threshold, F32).reshape(-1)[0])
    g5 = np.asarray(gauss_h, F32).reshape(-1)
    sob = np.asarray(sobel_h, F32).reshape(3, 3)
    diff = [float(sob[0, 0]), float(sob[0, 1]), float(sob[0, 2])]
    smooth = [float(sob[0, 0] / sob[0, 0]), float(sob[1, 0] / sob[0, 0]),
              float(sob[2, 0] / sob[0, 0])]
    g1r, g2r = float(g5[1] / g5[0]), float(g5[2] / g5[0])

    nc = _get_program(geo, g1r, g2r)
    in_maps = make_in_maps(geo, img, thr_val, g5, smooth, diff)
    global LAST_RESULTS
    kw = dict(TRACE_KWARGS) if TRACE else {}
    res = run_bass_kernel_spmd(nc, in_maps, core_ids=list(range(geo.n_cores)),
                               trace=TRACE, **kw)
    LAST_RESULTS = res
    return assemble(geo, res.results)


# test-harness hooks (the grader just calls kernel(**inputs))
TRACE = False
TRACE_KWARGS = {}
LAST_RESULTS = None


# revision 29
# speedup vs baseline: 1.1381x; 1.0812x over previous
"""CannyNet Trainium2 Bass kernel.

kernel(**inputs) takes the complete unsharded inputs (img [1,3,2048,2048],
threshold [1], gauss/sobel/dir filter tensors) and returns the reference
tuple (blurred_img, grad_mag, orient, thin_edges, thresholded,
early_threshold) computed on 8 NeuronCores.

Sharding: H is split spatially across the 8 cores (256 rows each).  Each
core receives a zero-padded slab of its rows (+4 halo rows, +4 halo cols)
and processes three 86-row "frames" that are fully self-contained: the
vertical convolutions are band matmuls on the tensor engine (the band
matrices encode the frame offset, the gaussian/sobel taps, and the exact
zero-padding behaviour of the reference at the global image edges), while
horizontal convolutions and pointwise math run on the vector, scalar and
gpsimd engines.  The per-pixel orientation sector (for NMS and the orient
output) is computed with sign tests against the 8 reference sector
boundaries instead of atan2.
"""

import numpy as np

try:  # the grading env has concourse importable; fall back to container path
    import concourse.bass as _b  # noqa: F401
except ImportError:  # pragma: no cover
    import sys
    for _p in ("/opt/trn_rl_repo",):
        if _p not in sys.path:
            sys.path.insert(0, _p)

import concourse.bass as bass
import concourse.bacc as bacc
import concourse.mybir as mybir
import concourse.tile as tile
from concourse.bass_utils import run_bass_kernel_spmd


# ----------------------------------------------------------------------------
# custom fused DVE ops (registered into the per-NEFF DVE table)
# ----------------------------------------------------------------------------
from concourse import dve_ops as _dve_ops
from concourse.dve_spec import (Spec as _Spec, Src0 as _S0, Src1 as _S1,
                                C0 as _C0, C1 as _C1, Zero as _Z,
                                select as _sel, sq as _sq, maxx as _maxx,
                                lower as _dve_lower, _has_src1 as _has_src1)
from concourse.dve_uop import DveOpSpec as _DveOpSpec
import numpy as _np


def _register_op(name, spec, subdim=False):
    for op in _dve_ops.OPS:
        if op.name == name:
            return op
    opcode = _dve_ops._CUSTOM_DVE_ROW_BASE + len(_dve_ops.OPS)
    assert opcode < 0x20, "custom DVE op row overflow"
    shas = {}
    for ver in ("v3", "v4"):
        s = _DveOpSpec(name=name, opcode=opcode, uops=_dve_lower(spec, ver=ver),
                       rd1_en=_has_src1(spec))
        shas[ver] = s.sha(ver)
    op = _dve_ops.DveOp(name, spec, subdim, shas)
    _dve_ops.OPS.append(op)
    _dve_ops._SUB_OPCODE_FOR_NAME[name] = opcode
    _dve_ops.CUSTOM_DVE_SPECS[name] = spec
    return op


OP_SQ2 = _register_op("CANNY_SQ2", _Spec(
    body=_sq(_S0) + _sq(_S1),
    reference=lambda in0, in1, s0, s1, imm2:
        ((in0 * in0) + (in1 * in1)).astype(_np.float32)))

OP_WEDGEM = _register_op("CANNY_WEDGEM", _Spec(
    body=((_S0 * _C0 + _S1) * _C1) > _Z,
    reference=lambda in0, in1, s0, s1, imm2:
        (((in0 * _np.float32(s0) + in1) * _np.float32(s1)) > 0)
        .astype(_np.float32)))

OP_IDXF = _register_op("CANNY_IDXF", _Spec(
    body=_sel(_S1 > _Z, _C0 - _S0, _S0),
    reference=lambda in0, in1, s0, s1, imm2:
        _np.where(in1 > 0, _np.float32(s0) - in0, in0).astype(_np.float32)))

OP_V4 = _register_op("CANNY_V4", _Spec(
    body=_S0 - _C0 * (_S0 >= _C1),
    reference=lambda in0, in1, s0, s1, imm2:
        (in0 - _np.float32(s0) * (in0 >= _np.float32(s1))).astype(_np.float32)))

OP_NMSTHIN = _register_op("CANNY_NMSTHIN", _Spec(
    body=_sel(_S1 < _S0, _S0, _Z),
    reference=lambda in0, in1, s0, s1, imm2:
        _np.where(in1 < in0, in0, 0.0).astype(_np.float32)))

OP_PMMASK = _register_op("CANNY_PMMASK", _Spec(
    body=_maxx(_S0 * _C0, _S1 * _C1),
    reference=lambda in0, in1, s0, s1, imm2:
        _np.maximum(in0 * _np.asarray(s0, _np.float32),
                    in1 * _np.asarray(s1, _np.float32)).astype(_np.float32)))


OP_CMPABS = _register_op("CANNY_CMPABS", _Spec(
    # (|Src0|*C0 < |Src1|) -> 1.0/0.0
    body=(_maxx(_S0, _Z - _S0) * _C0) < _maxx(_S1, _Z - _S1),
    reference=lambda in0, in1, s0, s1, imm2:
        ((_np.abs(in0) * _np.float32(s0)) < _np.abs(in1)).astype(_np.float32)))

OP_DSEL = _register_op("CANNY_DSEL", _Spec(
    # select(Src1 > 0, Src0, C0 - Src0)
    body=_sel(_S1 > _Z, _S0, _C0 - _S0),
    reference=lambda in0, in1, s0, s1, imm2:
        _np.where(in1 > 0, in0, _np.float32(s0) - in0).astype(_np.float32)))

OP_SIGNSEL = _register_op("CANNY_SIGNSEL", _Spec(
    # select(Src1 > 0, C0 + Src0, C0 - Src0)
    body=_sel(_S1 > _Z, _C0 + _S0, _C0 - _S0),
    reference=lambda in0, in1, s0, s1, imm2:
        _np.where(in1 > 0, _np.float32(s0) + in0,
                  _np.float32(s0) - in0).astype(_np.float32)))

F32 = np.float32
DT = mybir.dt.float32
OP = mybir.AluOpType
AF = mybir.ActivationFunctionType


class Geo:
    def __init__(self, H=2048, W=2048, n_cores=8, L=86, frame_offs=(0, 86, 170)):
        self.H, self.W, self.n_cores, self.L = H, W, n_cores, L
        self.rpc = H // n_cores
        self.frame_offs = list(frame_offs)          # relative frame starts
        assert self.frame_offs[-1] + L == self.rpc
        self.K = L + 8                               # A'/img frame rows
        self.K2 = L + 2                              # t1/t2/mag frame rows
        self.WP = W + 8                              # padded slab width
        self.n_frames = len(self.frame_offs)


# ----------------------------------------------------------------------------
# host-side prep (bands, masks, slabs)
# ----------------------------------------------------------------------------
def _make_bands(geo, g5, smooth, diff, frames):
    """Bands [K, M] f32: blur [K,L], t1/t2 [K,K2] per frame.
    A' partition k = global row o-4+k;  A' = A/g0 so fold g0 here.
    t1 = smooth_v(blur zero-padded at [0,H)), t2 = diff_v(same)."""
    g5p = np.asarray(g5, np.float64) * float(g5[0])
    H, L, K, K2 = geo.H, geo.L, geo.K, geo.K2
    out = []
    for o in frames:
        def blur_col(r):
            col = np.zeros(K)
            for i in range(5):
                k = (r + i - 2) - (o - 4)
                if 0 <= k < K:
                    col[k] = g5p[i]
            return col
        bb = np.zeros((K, L)); b1 = np.zeros((K, K2)); b2 = np.zeros((K, K2))
        for m in range(L):
            bb[:, m] = blur_col(o + m)
        for m in range(K2):
            r = o - 1 + m
            for j in range(3):
                rb = r + j - 1
                if 0 <= rb < H:
                    b1[:, m] += smooth[j] * blur_col(rb)
                    b2[:, m] += diff[j] * blur_col(rb)
        out.append((bb.astype(F32), b1.astype(F32), b2.astype(F32)))
    return out


def _make_rowmasks(geo, frames):
    """[K2, 2*n_frames]: col f*2+0 = mask_dn, f*2+1 = mask_up."""
    m = np.ones((geo.K2, 2 * len(frames)), F32)
    for fi, o in enumerate(frames):
        for p in range(geo.K2):
            if not (0 <= (o - 2 + p) < geo.H):
                m[p, 2 * fi + 0] = 0.0
            if not (0 <= (o + p) < geo.H):
                m[p, 2 * fi + 1] = 0.0
    return m


def _wedge_consts():
    out = []
    for k in range(4):
        b = (45.0 * k - 157.5) * 3.14159 / 180.0
        out.append((float(F32(-np.sin(b) / np.cos(b))), bool(np.cos(b) > 0)))
    return out


# ----------------------------------------------------------------------------
# the Bass program (SPMD; all boundary behaviour carried by input data)
# ----------------------------------------------------------------------------
def build_program(geo, g1r, g2r):
    nc = bacc.Bacc("TRN2", target_bir_lowering=False, debug=False)
    K, K2, L, W, WP, NF = geo.K, geo.K2, geo.L, geo.W, geo.WP, geo.n_frames
    HW = W // 2                                      # half width
    NQ = HW // 512 if HW >= 512 else 0               # 512-chunks per half

    slab = nc.declare_dram_parameter("slab", [3, geo.rpc + 8, WP], DT, isOutput=False)
    bandb_d = nc.declare_dram_parameter("bandb", [NF, K, L], DT, isOutput=False)
    band1_d = nc.declare_dram_parameter("band1", [NF, K, K2], DT, isOutput=False)
    band2_d = nc.declare_dram_parameter("band2", [NF, K, K2], DT, isOutput=False)
    rmask_d = nc.declare_dram_parameter("rmask", [K2, 2 * NF], DT, isOutput=False)
    thr_d = nc.declare_dram_parameter("thr", [K2, 1], DT, isOutput=False)

    blur_o = nc.declare_dram_parameter("blurred", [3, geo.rpc, W], DT, isOutput=True)
    grad_o = nc.declare_dram_parameter("grad", [geo.rpc, W], DT, isOutput=True)
    orient_o = nc.declare_dram_parameter("orient", [geo.rpc, W], DT, isOutput=True)
    thin_o = nc.declare_dram_parameter("thin", [geo.rpc, W], DT, isOutput=True)
    thresh_o = nc.declare_dram_parameter("thresh", [geo.rpc, W], DT, isOutput=True)
    early_o = nc.declare_dram_parameter("early", [geo.rpc, W], DT, isOutput=True)

    wedge = _wedge_consts()
    wedge_t1 = abs(wedge[0][0])
    wedge_t2 = abs(wedge[1][0])

    def psum_conv(pool, bnd, A, rows, half):
        """One half-width vertical band conv on PE into a PSUM tile."""
        pt = pool.tile([rows, HW], DT, tag="pt")
        cs = half * HW
        if NQ:
            for q in range(NQ):
                nc.tensor.matmul(pt[:, q * 512:(q + 1) * 512], bnd,
                                 A[:, cs + q * 512: cs + (q + 1) * 512],
                                 start=True, stop=True)
        else:
            nc.tensor.matmul(pt[:], bnd, A[:, cs:cs + HW], start=True, stop=True)
        return pt

    from contextlib import ExitStack
    with tile.TileContext(nc) as tc, ExitStack() as ctx:
        consts = ctx.enter_context(tc.tile_pool(name="consts", bufs=1))
        img_p = ctx.enter_context(tc.tile_pool(name="img", bufs=2))
        a_p = ctx.enter_context(tc.tile_pool(name="A", bufs=2))
        tmp_p = ctx.enter_context(tc.tile_pool(name="tmp", bufs=3))
        t12_p = ctx.enter_context(tc.tile_pool(name="t12", bufs=2))
        gxy_p = ctx.enter_context(tc.tile_pool(name="gxy", bufs=1))
        acc_p = ctx.enter_context(tc.tile_pool(name="acc", bufs=1))
        idx_p = ctx.enter_context(tc.tile_pool(name="idx", bufs=1))
        shift_p = ctx.enter_context(tc.tile_pool(name="shift", bufs=1))
        pm_p = ctx.enter_context(tc.tile_pool(name="pm", bufs=2))
        sel_p = ctx.enter_context(tc.tile_pool(name="sel", bufs=1))
        out_p = ctx.enter_context(tc.tile_pool(name="out", bufs=2))
        pb_p = ctx.enter_context(tc.tile_pool(name="pb", bufs=2, space="PSUM"))
        pt_p = ctx.enter_context(tc.tile_pool(name="pt", bufs=2, space="PSUM"))

        bandb = consts.tile([K, NF * L], DT)
        band1 = consts.tile([K, NF * K2], DT)
        band2 = consts.tile([K, NF * K2], DT)
        rmask = consts.tile([K2, 2 * NF], DT)
        thr = consts.tile([K2, 1], DT)
        for f in range(NF):
            nc.sync.dma_start(bandb[:, f * L:(f + 1) * L], bandb_d[f])
            nc.sync.dma_start(band1[:, f * K2:(f + 1) * K2], band1_d[f])
            nc.sync.dma_start(band2[:, f * K2:(f + 1) * K2], band2_d[f])
        nc.sync.dma_start(rmask[:], rmask_d[:])
        nc.sync.dma_start(thr[:], thr_d[:])

        for f, o in enumerate(geo.frame_offs):
            mag = acc_p.tile([K2, W + 4], DT, tag="mag")
            gxs = acc_p.tile([K2, W], DT, tag="gxs")
            gys = acc_p.tile([K2, W], DT, tag="gys")
            nc.gpsimd.memset(mag[:, 0:2], 0.0)
            nc.gpsimd.memset(mag[:, W + 2:W + 4], 0.0)

            for c in range(3):
                x = img_p.tile([K, WP], DT, tag="x")
                nc.sync.dma_start(x[:], slab[c, o:o + K, :])

                # horizontal 5-tap blur, scaled by 1/g0 (g0 folded into bands)
                u = tmp_p.tile([K, W], DT, tag="tmp")
                v = tmp_p.tile([K, W], DT, tag="tmp")
                nc.vector.tensor_tensor(u[:], x[:, 3:3 + W], x[:, 5:5 + W], OP.add)
                nc.vector.tensor_tensor(v[:], x[:, 2:2 + W], x[:, 6:6 + W], OP.add)
                w_t = tmp_p.tile([K, W], DT, tag="tmp")
                nc.vector.scalar_tensor_tensor(w_t[:], u[:], g1r, v[:], OP.mult, OP.add)
                A = a_p.tile([K, W], DT, tag="A")
                nc.vector.scalar_tensor_tensor(A[:], x[:, 4:4 + W], g2r, w_t[:],
                                               OP.mult, OP.add)

                # vertical band convs on PE
                bb = bandb[:, f * L:(f + 1) * L]
                b1 = band1[:, f * K2:(f + 1) * K2]
                b2 = band2[:, f * K2:(f + 1) * K2]
                blursb = out_p.tile([L, W], DT, tag="o")
                for half in range(2):
                    pb = psum_conv(pb_p, bb, A, L, half)
                    nc.scalar.copy(blursb[:, half * HW:(half + 1) * HW], pb[:])
                nc.sync.dma_start(blur_o[c, o:o + L, :], blursb[:])
                t1 = t12_p.tile([K2, W + 4], DT, tag="t12")
                t2 = t12_p.tile([K2, W + 4], DT, tag="t12")
                for t_sb, bnd in ((t1, b1), (t2, b2)):
                    nc.gpsimd.memset(t_sb[:, 0:2], 0.0)
                    nc.gpsimd.memset(t_sb[:, W + 2:W + 4], 0.0)
                    for half in range(2):
                        pt = psum_conv(pt_p, bnd, A, K2, half)
                        nc.scalar.copy(t_sb[:, 2 + half * HW:2 + (half + 1) * HW], pt[:])

                # sobel horizontal parts (channel 0 writes the gxs/gys
                # accumulators directly)
                gx = gxs if c == 0 else gxy_p.tile([K2, W], DT, tag="gx")
                nc.vector.scalar_tensor_tensor(gx[:], t1[:, 3:3 + W], -1.0,
                                               t1[:, 1:1 + W], OP.mult, OP.add)
                tt = tmp_p.tile([K2, W], DT, tag="tmp")
                nc.vector.tensor_tensor(tt[:], t2[:, 1:1 + W], t2[:, 3:3 + W], OP.add)
                gy = gys if c == 0 else gxy_p.tile([K2, W], DT, tag="gy")
                nc.vector.scalar_tensor_tensor(gy[:], t2[:, 2:2 + W], 2.0, tt[:],
                                               OP.mult, OP.add)

                # magnitude: msq = gx^2 + gy^2 in one fused DVE pass
                msq = tmp_p.tile([K2, W], DT, tag="tmp")
                nc.vector._custom_dve(OP_SQ2, out=msq[:], in0=gx[:], in1=gy[:])
                if c == 0:
                    nc.scalar.activation(mag[:, 2:2 + W], msq[:], AF.Sqrt)
                else:
                    mc = tmp_p.tile([K2, W], DT, tag="tmp")
                    nc.scalar.activation(mc[:], msq[:], AF.Sqrt)
                    nc.vector.tensor_tensor(mag[:, 2:2 + W], mag[:, 2:2 + W],
                                            mc[:], OP.add)
                    nc.vector.tensor_tensor(gxs[:], gxs[:], gx[:], OP.add)
                    nc.vector.tensor_tensor(gys[:], gys[:], gy[:], OP.add)

            # orientation sector via fused wedge sign tests
            mk = []
            for k, (ratio, cpos) in enumerate(wedge):
                m = tmp_p.tile([K2, W], DT, tag="tmp")
                nc.vector._custom_dve(OP_WEDGEM, out=m[:], in0=gxs[:],
                                      in1=gys[:], s0=ratio,
                                      s1=1.0 if cpos else -1.0)
                mk.append(m)
                if k == 2:  # free two tmp slots before the 4th mask
                    s01 = pm_p.tile([K2, W], DT, tag="pm")
                    nc.vector.tensor_tensor(s01[:], mk[0][:], mk[1][:], OP.add)
            s23 = pm_p.tile([K2, W], DT, tag="pm")
            nc.vector.tensor_tensor(s23[:], mk[2][:], mk[3][:], OP.add)
            S = idx_p.tile([K2, W], DT, tag="S")
            nc.vector.tensor_tensor(S[:], s01[:], s23[:], OP.add)
            idxf = idx_p.tile([K2, W], DT, tag="idxf")
            nc.vector._custom_dve(OP_IDXF, out=idxf[:], in0=S[:], in1=gys[:],
                                  s0=8.0)
            orient_t = out_p.tile([K2, W], DT, tag="o")
            nc.scalar.mul(orient_t[:], idxf[:], 45.0)
            nc.sync.dma_start(orient_o[o:o + L, :], orient_t[1:1 + L, :])

            # NMS
            magup = shift_p.tile([K2, W + 4], DT, tag="mu")
            magdn = shift_p.tile([K2, W + 4], DT, tag="md")
            nc.sync.dma_start(magup[0:K2 - 1, :], mag[1:K2, :])
            nc.sync.dma_start(magup[K2 - 1:K2, :], mag[K2 - 1:K2, :])
            nc.sync.dma_start(magdn[1:K2, :], mag[0:K2 - 1, :])
            nc.sync.dma_start(magdn[0:1, :], mag[0:1, :])

            V4 = idx_p.tile([K2, W], DT, tag="S")
            nc.vector._custom_dve(OP_V4, out=V4[:], in0=idxf[:], s0=4.0, s1=3.5)
            sel = sel_p.tile([K2, W], DT, tag="sel")
            nc.vector.tensor_tensor(sel[:], mag[:, 1:1 + W], mag[:, 3:3 + W], OP.max)
            mu_ap = rmask[:, 2 * f + 1:2 * f + 2]
            md_ap = rmask[:, 2 * f:2 * f + 1]
            for j, ta, ca, tb, cb in [(1, magup, 3, magdn, 1),
                                      (2, magup, 2, magdn, 2),
                                      (3, magup, 1, magdn, 3)]:
                pmj = pm_p.tile([K2, W], DT, tag="pm")
                nc.vector._custom_dve(OP_PMMASK, out=pmj[:],
                                      in0=ta[:, ca:ca + W], in1=tb[:, cb:cb + W],
                                      s0=mu_ap, s1=md_ap)
                mj = tmp_p.tile([K2, W], DT, tag="tmp")
                nc.vector.tensor_scalar(mj[:], V4[:], float(j), None, OP.is_equal)
                nc.vector.copy_predicated(sel[:], mj[:].bitcast(mybir.dt.int32),
                                          pmj[:])

            magc = mag[:, 2:2 + W]
            thin = out_p.tile([K2, W], DT, tag="o")
            nc.vector._custom_dve(OP_NMSTHIN, out=thin[:], in0=magc, in1=sel[:])
            thresh = out_p.tile([K2, W], DT, tag="o")
            nc.vector.scalar_tensor_tensor(thresh[:], thin[:], thr[:, 0:1], thin[:],
                                           OP.is_ge, OP.mult)
            early = out_p.tile([K2, W], DT, tag="o")
            nc.vector.scalar_tensor_tensor(early[:], magc, thr[:, 0:1], magc,
                                           OP.is_ge, OP.mult)
            nc.sync.dma_start(grad_o[o:o + L, :], mag[1:1 + L, 2:2 + W])
            nc.sync.dma_start(thin_o[o:o + L, :], thin[1:1 + L, :])
            nc.sync.dma_start(thresh_o[o:o + L, :], thresh[1:1 + L, :])
            nc.sync.dma_start(early_o[o:o + L, :], early[1:1 + L, :])
    nc.finalize()
    return nc


_prog_cache = {}


def _get_program(geo, g1r, g2r):
    key = (geo.H, geo.W, geo.n_cores, geo.L, tuple(geo.frame_offs),
           float(g1r), float(g2r))
    if key not in _prog_cache:
        _prog_cache[key] = build_program(geo, g1r, g2r)
    return _prog_cache[key]


def make_in_maps(geo, img, thr_val, g5, smooth, diff):
    H, W, rpc = geo.H, geo.W, geo.rpc
    padded = np.zeros((3, H + 8, W + 8), F32)
    padded[:, 4:H + 4, 4:W + 4] = img[0]
    in_maps = []
    for core in range(geo.n_cores):
        s = core * rpc
        frames = [s + fo for fo in geo.frame_offs]
        bands = _make_bands(geo, g5, smooth, diff, frames)
        in_maps.append(dict(
            slab=np.ascontiguousarray(padded[:, s:s + rpc + 8, :]),
            bandb=np.stack([b[0] for b in bands]),
            band1=np.stack([b[1] for b in bands]),
            band2=np.stack([b[2] for b in bands]),
            rmask=_make_rowmasks(geo, frames),
            thr=np.full((geo.K2, 1), thr_val, F32),
        ))
    return in_maps


def assemble(geo, outs):
    H, W, rpc = geo.H, geo.W, geo.rpc
    blurred = np.zeros((1, 3, H, W), F32)
    planes = [np.zeros((1, 1, H, W), F32) for _ in range(5)]
    names = ["grad", "orient", "thin", "thresh", "early"]
    for core in range(geo.n_cores):
        s = core * rpc
        r = outs[core]
        blurred[0, :, s:s + rpc] = r["blurred"]
        for p, n in zip(planes, names):
            p[0, 0, s:s + rpc] = r[n]
    return (blurred, planes[0], planes[1], planes[2], planes[3], planes[4])


def kernel(img, threshold, gauss_h, gauss_v, sobel_h, sobel_v, dir_w):
    geo = Geo()
    img = np.asarray(img, F32)
    thr_val = float(np.asarray(threshold, F32).reshape(-1)[0])
    g5 = np.asarray(gauss_h, F32).reshape(-1)
    sob = np.asarray(sobel_h, F32).reshape(3, 3)
    diff = [float(sob[0, 0]), float(sob[0, 1]), float(sob[0, 2])]
    smooth = [float(sob[0, 0] / sob[0, 0]), float(sob[1, 0] / sob[0, 0]),
              float(sob[2, 0] / sob[0, 0])]
    g1r, g2r = float(g5[1] / g5[0]), float(g5[2] / g5[0])

    nc = _get_program(geo, g1r, g2r)
    in_maps = make_in_maps(geo, img, thr_val, g5, smooth, diff)
    global LAST_RESULTS
    kw = dict(TRACE_KWARGS) if TRACE else {}
    res = run_bass_kernel_spmd(nc, in_maps, core_ids=list(range(geo.n_cores)),
                               trace=TRACE, **kw)
    LAST_RESULTS = res
    return assemble(geo, res.results)


# test-harness hooks (the grader just calls kernel(**inputs))
TRACE = False
TRACE_KWARGS = {}
LAST_RESULTS = None
